# revision 64
# baseline (speedup 1.0000x reference)
"""Trainium2 Bass kernel for nn_DomainAdaption (conv-conv-MoE-gated-residual).

Data-parallel over batch: 16 samples -> 8 NeuronCores, 2 samples/core.
Per sample on-device (conv3x3 = 9 shifted accumulated matmuls over a
host-padded [C,130,130] map, channels on partitions, 4-row x 128-col chunks):
  h1 = prelu(conv3x3(x, w1) + b1c)     float32r matmuls, ScalarE Prelu epilogue
  h2 = conv3x3(h1, w2) + b2c           bf16 matmuls, ScalarE epilogue + accum_out
  x1 = mean(h2, spatial)               accum_out columns + tensor_reduce
  a  = relu(W1[e] @ x1 + b1)           expert weights gathered on host
  g  = sigmoid(W2[e] @ a + b2)
  out = prelu(h2 * g + x)              sample 0: VectorE STT fusions (overlaps
                                       sample 1's convs); sample 1 (the tail):
                                       diag(g) & identity matmuls on the idle
                                       TensorE + ScalarE Prelu off PSUM
"""
import sys

if "/opt/trn_rl_repo" not in sys.path:
    sys.path.insert(0, "/opt/trn_rl_repo")

import numpy as np
import ml_dtypes

N, C, H, W = 16, 128, 128, 128
CH = 32
NCORES = 8
SPC = N // NCORES          # samples per core
HP = H + 2                 # padded rows/cols
NCHUNK = H // 4            # 32 chunks of 4 rows (512 spatial positions)
BF = ml_dtypes.bfloat16


def _build(prelu1: float, prelu2: float, conv_bias: bool):
    import concourse.mybir as mybir
    import concourse.tile as tile
    from concourse import bacc

    F32 = mybir.dt.float32
    F32R = mybir.dt.float32r
    BF16 = mybir.dt.bfloat16
    AF = mybir.ActivationFunctionType
    ALU = mybir.AluOpType

    nc = bacc.Bacc("TRN2", target_bir_lowering=False, debug=False,
                   num_devices=NCORES)

    # x arrives host-padded: [SPC, C, 130, 130] with the zero ring baked in
    x_d = nc.dram_tensor("x", [SPC, C, HP, HP], F32R, kind="ExternalInput").ap()
    cw1_d = nc.dram_tensor("cw1", [C, 9, C], F32R, kind="ExternalInput").ap()
    cw2_d = nc.dram_tensor("cw2", [C, 9, C], BF16, kind="ExternalInput").ap()
    c1b_d = nc.dram_tensor("c1b", [C, 1], F32, kind="ExternalInput").ap()
    c2b_d = nc.dram_tensor("c2b", [C, 1], F32, kind="ExternalInput").ap()
    w1t_d = nc.dram_tensor("w1t", [SPC, C, CH], F32, kind="ExternalInput").ap()
    b1_d = nc.dram_tensor("b1", [SPC, CH, 1], F32, kind="ExternalInput").ap()
    w2t_d = nc.dram_tensor("w2t", [SPC, CH, C], F32, kind="ExternalInput").ap()
    b2_d = nc.dram_tensor("b2", [SPC, C, 1], F32, kind="ExternalInput").ap()
    ident_d = nc.dram_tensor("ident", [C, C], F32R, kind="ExternalInput").ap()
    y_d = nc.dram_tensor("y", [SPC, C, H, W], F32, kind="ExternalOutput").ap()

    with tile.TileContext(nc) as tc, (
        tc.tile_pool(name="wp", bufs=1)) as wp, (
        tc.tile_pool(name="xp", bufs=1)) as xp, (
        tc.tile_pool(name="h1p", bufs=1)) as h1p, (
        tc.tile_pool(name="h2p", bufs=2)) as h2p, (
        tc.tile_pool(name="adw", bufs=2)) as adw, (
        tc.tile_pool(name="vec", bufs=2)) as vec, (
        tc.tile_pool(name="xrp", bufs=1)) as xrp, (
        tc.tile_pool(name="otp", bufs=3)) as otp, (
        tc.tile_pool(name="psc", bufs=6, space="PSUM")) as psc, (
        tc.tile_pool(name="psv", bufs=1, space="PSUM")) as psv:

        cw1_t = wp.tile([C, 9, C], F32R)
        c1b_t = wp.tile([C, 1], F32)
        cw2_t = wp.tile([C, 9, C], BF16)
        c2b_t = wp.tile([C, 1], F32)
        ident_t = wp.tile([C, C], F32R)

        # x in 4 row-band tiles (34 padded rows each, 2-row overlap) so conv1
        # chunks depend only on the band they read. Band k = padded rows
        # 32k .. 32k+33; full 130-col width with host-baked zero ring.
        xb = [xp.tile([C, 34, HP], F32R, name=f"xb{k}") for k in range(4)]

        h1_pad = h1p.tile([C, HP, HP], BF16)
        nc.vector.memset(h1_pad[:, 0, :], 0)
        nc.vector.memset(h1_pad[:, HP - 1, :], 0)
        nc.vector.memset(h1_pad[:, 1:HP - 1, 0], 0)
        nc.vector.memset(h1_pad[:, 1:HP - 1, HP - 1], 0)

        for s in range(SPC):
            # conv1 (float32r) -> prelu -> h1_pad; banded x loads so chunk c
            # only waits on band c//8. Weights stream in behind band 0 so the
            # first matmul isn't queued behind them on the serial DMA path.
            for k in range(4):
                if s == 0 and k == 0:
                    nc.scalar.dma_start(cw1_t[:], cw1_d)
                    nc.scalar.dma_start(c1b_t[:], c1b_d)
                    nc.sync.dma_start(xb[k][:, 0:7], x_d[s, :, 0:7, :])
                    nc.sync.dma_start(xb[k][:, 7:18], x_d[s, :, 7:18, :])
                else:
                    nc.sync.dma_start(xb[k][:, 0:18],
                                      x_d[s, :, 32 * k:32 * k + 18, :])
                nc.sync.dma_start(xb[k][:, 18:34],
                                  x_d[s, :, 32 * k + 18:32 * k + 34, :])
                if s == 0 and k == 1:
                    nc.sync.dma_start(cw2_t[:], cw2_d)
                    nc.sync.dma_start(c2b_t[:], c2b_d)
                    nc.sync.dma_start(ident_t[:], ident_d)
                for c in range(8 * k, 8 * k + 8):
                    pch = psc.tile([C, 4, W], F32)
                    lr = 4 * (c - 8 * k)
                    for t in range(9):
                        dy, dx = t // 3, t % 3
                        nc.tensor.matmul(
                            pch[:], cw1_t[:, t, :],
                            xb[k][:, lr + dy:lr + dy + 4, dx:dx + W],
                            start=(t == 0), stop=(t == 8))
                    nc.scalar.activation(
                        h1_pad[:, 4 * c + 1:4 * c + 5, 1:W + 1], pch[:],
                        AF.Prelu, bias=(c1b_t[:] if conv_bias else 0.0),
                        alpha=prelu1)

            # conv2 (bf16) -> h2 + pooling partials (epilogue on ScalarE so
            # VectorE stays free for the previous sample's gated residual)
            h2 = h2p.tile([C, NCHUNK // 2, 8, W], BF16)
            stats = h2p.tile([C, NCHUNK], F32)
            for c in range(NCHUNK):
                pch = psc.tile([C, 4, W], F32)
                for t in range(9):
                    dy, dx = t // 3, t % 3
                    nc.tensor.matmul(
                        pch[:], cw2_t[:, t, :],
                        h1_pad[:, 4 * c + dy:4 * c + dy + 4, dx:dx + W],
                        start=(t == 0), stop=(t == 8))
                nc.scalar.activation(
                    h2[:, c // 2, (c % 2) * 4:(c % 2) * 4 + 4, :], pch[:],
                    AF.Identity, bias=(c2b_t[:] if conv_bias else 0.0),
                    accum_out=stats[:, c:c + 1])

            # per-sample expert (host-gathered) adapter weights
            w1t_t = adw.tile([C, CH], F32)
            nc.sync.dma_start(w1t_t[:], w1t_d[s])
            b1_t = adw.tile([CH, 1], F32)
            nc.sync.dma_start(b1_t[:], b1_d[s])
            w2t_t = adw.tile([CH, C], F32)
            nc.sync.dma_start(w2t_t[:], w2t_d[s])
            b2_t = adw.tile([C, 1], F32)
            nc.sync.dma_start(b2_t[:], b2_d[s])

            # global mean -> adapter MLP -> sigmoid gate
            x1 = vec.tile([C, 1], F32)
            nc.vector.tensor_reduce(x1[:], stats[:], axis=mybir.AxisListType.X,
                                    op=ALU.add)
            psa = psv.tile([CH, 1], F32)
            nc.tensor.matmul(psa[:], w1t_t[:], x1[:], start=True, stop=True)
            a_t = vec.tile([CH, 1], F32)
            nc.vector.tensor_scalar(a_t[:], psa[:], b1_t[:], 0.0,
                                    ALU.add, ALU.max)
            psg = psv.tile([C, 1], F32)
            nc.tensor.matmul(psg[:], w2t_t[:], a_t[:], start=True, stop=True)
            gate = vec.tile([C, 1], F32)
            nc.scalar.activation(gate[:], psg[:], AF.Sigmoid, bias=b2_t[:])

            # out = prelu(h2 * gate + x)
            if s == SPC - 1:
                # Tail sample: PE is idle now, so gate via a diagonal matmul
                # accumulated with identity @ x (x read from the resident f32r
                # bands), then Prelu straight off PSUM on ScalarE.
                diag_t = vec.tile([C, C], BF16)
                nc.vector.tensor_scalar_mul(
                    diag_t[:], ident_t[:].bitcast(F32), gate[:])
                for c in range(NCHUNK):
                    pch = psc.tile([C, 4, W], F32)
                    nc.tensor.matmul(
                        pch[:], diag_t[:],
                        h2[:, c // 2, (c % 2) * 4:(c % 2) * 4 + 4, :],
                        start=True, stop=False)
                    nc.tensor.matmul(
                        pch[:], ident_t[:],
                        xb[c // 8][:, 4 * (c % 8) + 1:4 * (c % 8) + 5,
                                   1:W + 1],
                        start=False, stop=True)
                    o_t = otp.tile([C, 4, W], F32, bufs=10, tag="ot")
                    nc.scalar.activation(o_t[:], pch[:], AF.Prelu,
                                         alpha=prelu2)
                    # alternate store-issue queues so no sequencer paces the
                    # tail (ScalarE only runs the 570ns Prelus)
                    if c % 2 == 0:
                        nc.sync.dma_start(y_d[s, :, 4 * c:4 * c + 4, :],
                                          o_t[:])
                    else:
                        nc.gpsimd.dma_start(y_d[s, :, 4 * c:4 * c + 4, :],
                                            o_t[:])
            else:
                # Overlapped sample: gated residual on VectorE while PE runs
                # the next sample's convs; residual re-streamed from HBM.
                for b in range(NCHUNK // 2):
                    xr = xrp.tile([C, 8, HP], F32)
                    nc.gpsimd.dma_start(
                        xr[:], x_d[s, :, 8 * b + 1:8 * b + 9, :].bitcast(F32))
                    for j in range(2):
                        c = 2 * b + j
                        t_t = otp.tile([C, 4, W], F32, bufs=10, tag="ot")
                        nc.vector.scalar_tensor_tensor(
                            t_t[:], h2[:, b, 4 * j:4 * j + 4, :], gate[:],
                            xr[:, 4 * j:4 * j + 4, 1:W + 1],
                            op0=ALU.mult, op1=ALU.add)
                        o_t = otp.tile([C, 4, W], F32, bufs=10, tag="ot")
                        # prelu(t) = max(a*t, t), valid for 0<=a<=1; on DVE
                        # so ScalarE only runs conv2 epilogues during the
                        # next sample's conv2 phase
                        if 0.0 <= prelu2 <= 1.0:
                            nc.vector.scalar_tensor_tensor(
                                o_t[:], t_t[:], prelu2, t_t[:],
                                op0=ALU.mult, op1=ALU.max)
                        else:
                            nc.scalar.activation(o_t[:], t_t[:], AF.Prelu,
                                                 alpha=prelu2)
                        nc.sync.dma_start(y_d[s, :, 4 * c:4 * c + 4, :],
                                          o_t[:])

    nc.compile()
    return nc


_CACHE = {}


def _get_program(prelu1, prelu2, conv_bias):
    key = (float(prelu1), float(prelu2), bool(conv_bias))
    if key not in _CACHE:
        _CACHE[key] = _build(*key)
    return _CACHE[key]


def _prep(x, intensity, conv1_w, conv1_b, prelu1, conv2_w, conv2_b,
          aW1, ab1, aW2, ab2, prelu2):
    x = np.asarray(x, np.float32)
    idx = np.asarray(intensity).astype(np.int64) - 1
    conv1_w = np.asarray(conv1_w, np.float32)
    conv1_b = np.asarray(conv1_b, np.float32)
    conv2_w = np.asarray(conv2_w, np.float32)
    conv2_b = np.asarray(conv2_b, np.float32)
    aW1 = np.asarray(aW1, np.float32)
    ab1 = np.asarray(ab1, np.float32)
    aW2 = np.asarray(aW2, np.float32)
    ab2 = np.asarray(ab2, np.float32)

    # [Co,Ci,ky,kx] -> [Ci, tap, Co] so lhsT slices are [K=Ci, M=Co]
    cw1 = np.ascontiguousarray(conv1_w.transpose(1, 2, 3, 0).reshape(C, 9, C))
    cw2 = np.ascontiguousarray(
        conv2_w.transpose(1, 2, 3, 0).reshape(C, 9, C)).astype(BF)
    # per-sample expert gather; fold the 1/(H*W) mean into W1
    w1t = np.ascontiguousarray(
        (aW1[idx] / float(H * W)).transpose(0, 2, 1))      # [N, C, CH]
    b1g = np.ascontiguousarray(ab1[idx])[:, :, None]       # [N, CH, 1]
    w2t = np.ascontiguousarray(aW2[idx].transpose(0, 2, 1))  # [N, CH, C]
    b2g = np.ascontiguousarray(ab2[idx])[:, :, None]       # [N, C, 1]

    conv_bias = bool(np.any(conv1_b) or np.any(conv2_b))
    nc = _get_program(float(prelu1), float(prelu2), conv_bias)

    xpad = np.zeros((N, C, HP, HP), np.float32)
    xpad[:, :, 1:H + 1, 1:W + 1] = x

    in_maps = []
    for i in range(NCORES):
        sl = slice(i * SPC, (i + 1) * SPC)
        in_maps.append(dict(
            x=xpad[sl], cw1=cw1, cw2=cw2,
            c1b=conv1_b[:, None], c2b=conv2_b[:, None],
            w1t=w1t[sl], b1=b1g[sl], w2t=w2t[sl], b2=b2g[sl],
            ident=np.eye(C, dtype=np.float32)))
    return nc, in_maps


def kernel(**inputs):
    import time
    from concourse.bass_utils import run_bass_kernel_spmd

    nc, in_maps = _prep(**inputs)
    res = None
    for attempt, pause in enumerate((0, 15, 60, 120)):
        if pause:
            time.sleep(pause)
        try:
            res = run_bass_kernel_spmd(nc, in_maps,
                                       core_ids=list(range(NCORES)))
            break
        except Exception:
            # transient NRT_EXEC_UNIT_UNRECOVERABLE (wedged core); retry
            if attempt == 3:
                raise
    return np.concatenate([r["y"] for r in res.results], axis=0)
